# revision 54
# baseline (speedup 1.0000x reference)
"""Local self-attention (window=65) Trainium2 kernel, 8 NeuronCores.

Sharding: 4096 tokens (B*S flattened) split into 8 shards of 512 tokens.
Each core gets a halo'd, pre-transposed x slice plus replicated weights
(halo = 32 tokens each side, zero-padded at batch-sequence edges; zero x
tokens produce exactly-zero k/v since the qkv projection has no bias,
matching the reference's zero-padding semantics).

Per-core pipeline (Bass/Tile, bf16 matmuls with fp32 accumulation):
  1. v projection (token-major) plus partition-shifted copies (vdn, v64)
     that line the unaligned 64-key chunks up with the right PSUM
     partitions; qT/kT projections (feature-major) are emitted head-pair
     by head-pair, interleaved between attention groups so the PE stream
     stays dense and the HAM clock stays at 8/8.
  2. Per (head-pair, block): both heads' banded scores [128,192] into one
     2-bank PSUM tile, one ACT exp for the pair, band-mask fused with the
     row-sum in one DVE scalar_tensor_tensor per head, 2-op normalize
     into the pair tile L, 3 PE transposes, then 4 AV matmuls back into
     feature-major layout (col-packed per pair).
  3. Output projection + bias per block at the end, DMA out.
"""

import numpy as np
import ml_dtypes

import concourse.bass as bass
import concourse.mybir as mybir
import concourse.tile as tile
from concourse import bacc
from concourse.bass_utils import run_bass_kernel_spmd

F32 = mybir.dt.float32
BF16 = mybir.dt.bfloat16

# ---- problem constants (hardcoded) ----
B, S, DM = 2, 2048, 512
H, D, WIN = 8, 64, 65
PAD = WIN // 2              # 32
NCORES = 8
SHARD = B * S // NCORES     # 512 tokens per core
HALO = SHARD + 2 * PAD      # 576
NBLK = SHARD // 128         # 4 query blocks
KEYS = 128 + 2 * PAD        # 192 keys per block
NFT = DM // 128             # 4 feature tiles

DT_PROJ = BF16
DT_ATTN = BF16


def _np_dt(dt):
    return {F32: np.float32, BF16: ml_dtypes.bfloat16}[dt]


def _build_program(stage="full"):
    nc = bacc.Bacc("TRN2", target_bir_lowering=False, debug=False)

    # tile-packed layouts: [128, nft*cols] with feature-tile kc at
    # cols [kc*cols, (kc+1)*cols) -- one DMA per matrix (dma_start issue
    # costs ~600ns of sequencer time each, so fewer is better)
    xT_d = nc.dram_tensor("xT", [128, NFT * HALO], DT_PROJ,
                          kind="ExternalInput")
    Wq_d = nc.dram_tensor("Wq", [128, NFT * DM], DT_PROJ,
                          kind="ExternalInput")
    Wk_d = nc.dram_tensor("Wk", [128, NFT * DM], DT_PROJ,
                          kind="ExternalInput")
    Wv_d = nc.dram_tensor("Wv", [128, NFT * DM], DT_PROJ,
                          kind="ExternalInput")
    Wo_d = nc.dram_tensor("Wo", [128, NFT * DM], DT_ATTN,
                          kind="ExternalInput")
    bias_d = nc.dram_tensor("bias", [DM], F32, kind="ExternalInput")
    mi_d = nc.dram_tensor("maskident", [128, KEYS + 128], DT_ATTN,
                          kind="ExternalInput")
    out_d = nc.dram_tensor("out", [SHARD, DM], F32, kind="ExternalOutput")

    Exp = mybir.ActivationFunctionType.Exp
    Copy = mybir.ActivationFunctionType.Copy
    Mult = mybir.AluOpType.mult

    cp_idx = [0]

    def copy_alt(out, in_):
        # alternate PSUM->SBUF copies between DVE and ACT to balance load
        if cp_idx[0] % 2 == 0:
            nc.vector.tensor_copy(out=out, in_=in_)
        else:
            nc.scalar.activation(out=out, in_=in_, func=Copy)
        cp_idx[0] += 1

    with tile.TileContext(nc) as tc:
        with (
            tc.tile_pool(name="consts", bufs=1) as cpool,
            tc.tile_pool(name="qkv", bufs=1) as qpool,
            tc.tile_pool(name="work", bufs=4) as wpool,
            tc.tile_pool(name="outp", bufs=2) as opool,
            tc.tile_pool(name="pp", bufs=2, space="PSUM") as pp,
            tc.tile_pool(name="ps", bufs=2, space="PSUM") as ps,
            tc.tile_pool(name="pw", bufs=2, space="PSUM") as pw,
        ):
            pa = pw

            # ---- load constants (one DMA per matrix) ----
            xT_all = cpool.tile([128, NFT * HALO], DT_PROJ, tag="xT")
            nc.sync.dma_start(out=xT_all[:], in_=xT_d[:, :])
            Wv_all = cpool.tile([128, NFT * DM], DT_PROJ, tag="Wv")
            nc.sync.dma_start(out=Wv_all[:], in_=Wv_d[:, :])
            Wq_all = cpool.tile([128, NFT * DM], DT_PROJ, tag="Wq")
            nc.sync.dma_start(out=Wq_all[:], in_=Wq_d[:, :])
            Wk_all = cpool.tile([128, NFT * DM], DT_PROJ, tag="Wk")
            nc.sync.dma_start(out=Wk_all[:], in_=Wk_d[:, :])
            Wo_all = cpool.tile([128, NFT * DM], DT_ATTN, tag="Wo")
            nc.sync.dma_start(out=Wo_all[:], in_=Wo_d[:, :])
            mi_sb = cpool.tile([128, KEYS + 128], DT_ATTN, tag="mi")
            nc.sync.dma_start(out=mi_sb[:], in_=mi_d[:, :])
            bias_sb = cpool.tile([128, DM], F32, tag="bias")
            bias_ap = bias_d[:]
            nc.gpsimd.dma_start(
                out=bias_sb[:],
                in_=bass.AP(tensor=bias_ap.tensor, offset=bias_ap.offset,
                            ap=[[0, 128]] + list(bias_ap.ap)),
            )
            xT_sb = [xT_all[:, kc * HALO:(kc + 1) * HALO]
                     for kc in range(NFT)]
            Wq_sb = [Wq_all[:, kc * DM:(kc + 1) * DM] for kc in range(NFT)]
            Wk_sb = [Wk_all[:, kc * DM:(kc + 1) * DM] for kc in range(NFT)]
            Wv_sb = [Wv_all[:, kc * DM:(kc + 1) * DM] for kc in range(NFT)]
            Wo_sb = [Wo_all[:, kc * DM:(kc + 1) * DM] for kc in range(NFT)]
            mask_sb = mi_sb[:, 0:KEYS]
            ident_sb = mi_sb[:, KEYS:KEYS + 128]

            # ---- v projection (token-major) + shifted copies ----
            v_sb = [None] * 5
            v_dn = [None] * NBLK   # vdn[t][64:128] = v[t][0:64]
            v_64 = [None] * NBLK   # v64[t] = tokens [128t+64, 128t+192)

            def emit_v(tt):
                rows = 128 if tt < 4 else HALO - 512
                psv = pp.tile([128, DM], F32, tag="pp", name="psv")
                for kc in range(NFT):
                    nc.tensor.matmul(
                        psv[:rows, :], xT_sb[kc][:, tt * 128:tt * 128 + rows],
                        Wv_sb[kc][:, :],
                        start=(kc == 0), stop=(kc == NFT - 1))
                vt = qpool.tile([128, DM], DT_ATTN, tag=f"v{tt}",
                                name=f"v{tt}")
                copy_alt(vt[:rows, :], psv[:rows, :])
                v_sb[tt] = vt
                if tt < NBLK:
                    vd = qpool.tile([128, DM], DT_ATTN, tag=f"vdn{tt}",
                                    name=f"vdn{tt}")
                    nc.gpsimd.dma_start(out=vd[64:128, :], in_=vt[0:64, :])
                    v_dn[tt] = vd
                if tt >= 1:
                    v6 = qpool.tile([128, DM], DT_ATTN, tag=f"v64{tt - 1}",
                                    name=f"v64{tt - 1}")
                    nc.gpsimd.dma_start(out=v6[0:64, :],
                                        in_=v_sb[tt - 1][64:128, :])
                    nc.gpsimd.dma_start(out=v6[64:128, :], in_=vt[0:64, :])
                    v_64[tt - 1] = v6

            # ---- qT/kT projection for one head pair (feature tile) ----
            qT_sb = [None] * NFT
            kT_sb = [None] * NFT

            def emit_qk(ft):
                csl = slice(ft * 128, ft * 128 + 128)
                psq = pp.tile([128, SHARD], F32, tag="pp", name="psq")
                for kc in range(NFT):
                    nc.tensor.matmul(
                        psq[:], Wq_sb[kc][:, csl], xT_sb[kc][:, PAD:PAD + SHARD],
                        start=(kc == 0), stop=(kc == NFT - 1))
                qt = qpool.tile([128, SHARD], DT_ATTN, tag=f"qT{ft}",
                                name=f"qT{ft}")
                copy_alt(qt[:], psq[:])
                qT_sb[ft] = qt

                kt = qpool.tile([128, HALO], DT_ATTN, tag=f"kT{ft}",
                                name=f"kT{ft}")
                psk = pp.tile([128, SHARD], F32, tag="pp", name="psk")
                for kc in range(NFT):
                    nc.tensor.matmul(
                        psk[:], Wk_sb[kc][:, csl], xT_sb[kc][:, 0:512],
                        start=(kc == 0), stop=(kc == NFT - 1))
                copy_alt(kt[:, 0:512], psk[:])
                psk2 = pp.tile([128, 64], F32, tag="pp", name="psk2")
                for kc in range(NFT):
                    nc.tensor.matmul(
                        psk2[:], Wk_sb[kc][:, csl], xT_sb[kc][:, 512:HALO],
                        start=(kc == 0), stop=(kc == NFT - 1))
                copy_alt(kt[:, 512:HALO], psk2[:])
                kT_sb[ft] = kt

            attnT_sb = [qpool.tile([128, SHARD], DT_ATTN, tag=f"attnT{i}",
                                   name=f"attnT{i}")
                        for i in range(NFT)]

            # ---- attention stages ----
            # L = [w0 (keys 0:192) | w1 (keys 0:192)] contiguous; transposes:
            #   wT[:,0:128]   = L[:,0:128].T   = h0 keys 0:128
            #   wT[:,128:256] = L[:,128:256].T = h0 keys 128:192 @ p0:64
            #                                  + h1 keys 0:64    @ p64:128
            #   wT[:,256:384] = L[:,256:384].T = h1 keys 64:192
            pendingB = []
            SKEW = 1

            def flushB(n=0):
                while len(pendingB) > n:
                    pendingB.pop(0)()

            def stageA(b, hp):
                qsl = slice(b * 128, b * 128 + 128)
                ksl = slice(b * 128, b * 128 + KEYS)
                denom = wpool.tile([128, 2], F32, tag="denom")
                sc = ps.tile([128, 1024], F32, tag="ps", name="sc")
                for j in range(2):
                    rsl = slice(j * 64, j * 64 + 64)
                    nc.tensor.matmul(
                        sc[:, 512 * j:512 * j + KEYS],
                        qT_sb[hp][rsl, qsl], kT_sb[hp][rsl, ksl],
                        start=True, stop=True)
                e = wpool.tile([128, 2 * KEYS], DT_ATTN, tag="e", name="e")
                sc_view = sc[:].rearrange(
                    "p (h x) -> p h x", h=2)[:, :, 0:KEYS]
                nc.scalar.activation(
                    out=e[:].rearrange("p (h k) -> p h k", h=2),
                    in_=sc_view, func=Exp, scale=0.125)
                ems = []
                for j in range(2):
                    em = wpool.tile([128, KEYS], DT_ATTN, tag="em",
                                    name=f"em{j}")
                    nc.vector.scalar_tensor_tensor(
                        out=em[:], in0=e[:, j * KEYS:(j + 1) * KEYS],
                        scalar=1.0, in1=mask_sb[:],
                        op0=Mult, op1=Mult,
                        accum_out=denom[:, j:j + 1])
                    ems.append(em)
                recip = wpool.tile([128, 2], F32, tag="recip")
                nc.vector.reciprocal(recip[:], denom[:])
                L = wpool.tile([128, 384], DT_ATTN, tag="L", name="L")
                for j in range(2):
                    nc.vector.tensor_scalar_mul(
                        L[:, 192 * j:192 * (j + 1)],
                        ems[j][:], recip[:, j:j + 1])
                return L

            def stageB(b, hp, L):
                qsl = slice(b * 128, b * 128 + 128)
                pwt = pw.tile([128, 384], DT_ATTN, tag="pw", name="pwt")
                nc.tensor.transpose(pwt[:, 0:128], L[:, 0:128], ident_sb[:])
                nc.tensor.transpose(pwt[:, 128:256], L[:, 128:256],
                                    ident_sb[:])
                nc.tensor.transpose(pwt[:, 256:384], L[:, 256:384],
                                    ident_sb[:])
                wT = wpool.tile([128, 384], DT_ATTN, tag="wT", name="wT")
                copy_alt(wT[:], pwt[:])
                h0 = slice(2 * hp * 64, 2 * hp * 64 + 64)
                h1 = slice((2 * hp + 1) * 64, (2 * hp + 1) * 64 + 64)
                pav = pa.tile([128, 128], F32, tag="pw")
                nc.tensor.matmul(pav[0:64, :], v_sb[b][:, h0],
                                 wT[:, 0:128], start=True, stop=False)
                nc.tensor.matmul(pav[0:64, :], v_sb[b + 1][0:64, h0],
                                 wT[0:64, 128:256], start=False, stop=True)
                nc.tensor.matmul(pav[64:128, :], v_dn[b][64:128, h1],
                                 wT[64:128, 128:256], start=True, stop=False)
                nc.tensor.matmul(pav[64:128, :], v_64[b][:, h1],
                                 wT[:, 256:384], start=False, stop=True)
                copy_alt(attnT_sb[hp][:, qsl], pav[:])

            # ---- main schedule ----
            for tt in range(5):
                emit_v(tt)
            emit_qk(0)
            if stage == "proj":
                for ft in range(1, NFT):
                    emit_qk(ft)
                for tt in range(NBLK):
                    osb = opool.tile([128, DM], F32, tag="osb")
                    nc.vector.tensor_copy(osb[:], v_sb[tt][:])
                    nc.sync.dma_start(
                        out=out_d[tt * 128:(tt + 1) * 128, :], in_=osb[:])
            else:
                for hp in range(H // 2):
                    for b in range(NBLK):
                        L = stageA(b, hp)
                        flushB(SKEW - 1)
                        pendingB.append(lambda b=b, hp=hp, L=L:
                                        stageB(b, hp, L))
                        if b == 1 and hp < NFT - 1:
                            emit_qk(hp + 1)
                flushB()
                for b in range(NBLK):
                    qsl = slice(b * 128, b * 128 + 128)
                    po = pp.tile([128, DM], F32, tag="pp")
                    for kc in range(NFT):
                        nc.tensor.matmul(
                            po[:], attnT_sb[kc][:, qsl], Wo_sb[kc][:, :],
                            start=(kc == 0), stop=(kc == NFT - 1))
                    osb = opool.tile([128, DM], F32, tag="osb")
                    nc.vector.tensor_add(osb[:], po[:], bias_sb[:])
                    nc.sync.dma_start(out=out_d[b * 128:(b + 1) * 128, :],
                                      in_=osb[:])

    nc.compile()
    return nc


_CACHE = {}


def _get_program():
    if "nc" not in _CACHE:
        _CACHE["nc"] = _build_program()
    return _CACHE["nc"]


def _make_in_maps(x, W_qkv, W_out, b_out):
    np_proj = _np_dt(DT_PROJ)
    np_attn = _np_dt(DT_ATTN)
    def pack(M, dt):
        # [512, C] -> [128, 4*C] with row-tile kc at cols [kc*C, (kc+1)*C)
        return np.ascontiguousarray(
            np.concatenate([M[128 * kc:128 * (kc + 1)] for kc in range(NFT)],
                           axis=1), dtype=dt)

    Wr = W_qkv.reshape(DM, H, 3, D)
    Wq = pack(Wr[:, :, 0, :].reshape(DM, DM), np_proj)
    Wk = pack(Wr[:, :, 1, :].reshape(DM, DM), np_proj)
    Wv = pack(Wr[:, :, 2, :].reshape(DM, DM), np_proj)
    Wo = pack(W_out, np_attn)
    bias = np.ascontiguousarray(b_out, dtype=np.float32)
    ii = np.arange(128)[:, None]
    kk = np.arange(KEYS)[None, :]
    mask2 = np.where((kk >= ii) & (kk <= ii + WIN - 1), 1.0, 0.0)
    maskident = np.ascontiguousarray(
        np.concatenate([mask2, np.eye(128)], axis=1), dtype=np_attn)

    in_maps = []
    for c in range(NCORES):
        bidx, s0 = c // (NCORES // B), (c % (NCORES // B)) * SHARD
        xh = np.zeros((HALO, DM), np.float32)
        lo, hi = s0 - PAD, s0 + SHARD + PAD
        clo, chi = max(lo, 0), min(hi, S)
        xh[clo - lo:chi - lo] = x[bidx, clo:chi]
        xT = pack(np.ascontiguousarray(xh.T), np_proj)
        in_maps.append({
            "xT": xT, "Wq": Wq, "Wk": Wk, "Wv": Wv, "Wo": Wo,
            "bias": bias, "maskident": maskident,
        })
    return in_maps


def kernel(x, W_qkv, W_out, b_out, _trace=False, _tmpdir=None):
    x = np.asarray(x, dtype=np.float32)
    W_qkv = np.asarray(W_qkv, dtype=np.float32)
    W_out = np.asarray(W_out, dtype=np.float32)
    b_out = np.asarray(b_out, dtype=np.float32)

    nc = _get_program()
    in_maps = _make_in_maps(x, W_qkv, W_out, b_out)
    res = run_bass_kernel_spmd(
        nc, in_maps, list(range(NCORES)), trace=_trace, tmpdir=_tmpdir)
    _CACHE["last_results"] = res
    out = np.concatenate(
        [res.results[c]["out"] for c in range(NCORES)], axis=0)
    return out.reshape(B, S, DM).astype(np.float32)


# revision 55
# speedup vs baseline: 1.0204x; 1.0204x over previous
"""Local self-attention (window=65) Trainium2 kernel, 8 NeuronCores.

Sharding: 4096 tokens (B*S flattened) split into 8 shards of 512 tokens.
Each core gets a halo'd, pre-transposed x slice plus replicated weights
(halo = 32 tokens each side, zero-padded at batch-sequence edges; zero x
tokens produce exactly-zero k/v since the qkv projection has no bias,
matching the reference's zero-padding semantics).

Per-core pipeline (Bass/Tile, bf16 matmuls with fp32 accumulation):
  1. v projection (token-major) plus partition-shifted copies (vdn, v64)
     that line the unaligned 64-key chunks up with the right PSUM
     partitions; qT/kT projections (feature-major) are emitted head-pair
     by head-pair, interleaved between attention groups so the PE stream
     stays dense and the HAM clock stays at 8/8.
  2. Per (head-pair, block): both heads' banded scores [128,192] into one
     2-bank PSUM tile, one ACT exp for the pair, band-mask fused with the
     row-sum in one DVE scalar_tensor_tensor per head, 2-op normalize
     into the pair tile L, 3 PE transposes, then 4 AV matmuls back into
     feature-major layout (col-packed per pair).
  3. Output projection + bias per block at the end, DMA out.
"""

import numpy as np
import ml_dtypes

import concourse.bass as bass
import concourse.mybir as mybir
import concourse.tile as tile
from concourse import bacc
from concourse.bass_utils import run_bass_kernel_spmd

F32 = mybir.dt.float32
BF16 = mybir.dt.bfloat16

# ---- problem constants (hardcoded) ----
B, S, DM = 2, 2048, 512
H, D, WIN = 8, 64, 65
PAD = WIN // 2              # 32
NCORES = 8
SHARD = B * S // NCORES     # 512 tokens per core
HALO = SHARD + 2 * PAD      # 576
NBLK = SHARD // 128         # 4 query blocks
KEYS = 128 + 2 * PAD        # 192 keys per block
NFT = DM // 128             # 4 feature tiles

DT_PROJ = BF16
DT_ATTN = BF16


def _np_dt(dt):
    return {F32: np.float32, BF16: ml_dtypes.bfloat16}[dt]


def _build_program(stage="full"):
    nc = bacc.Bacc("TRN2", target_bir_lowering=False, debug=False)

    # tile-packed layouts: [128, nft*cols] with feature-tile kc at
    # cols [kc*cols, (kc+1)*cols) -- one DMA per matrix (dma_start issue
    # costs ~600ns of sequencer time each, so fewer is better)
    xT_d = nc.dram_tensor("xT", [128, NFT * HALO], DT_PROJ,
                          kind="ExternalInput")
    Wq_d = nc.dram_tensor("Wq", [128, NFT * DM], DT_PROJ,
                          kind="ExternalInput")
    Wk_d = nc.dram_tensor("Wk", [128, NFT * DM], DT_PROJ,
                          kind="ExternalInput")
    Wv_d = nc.dram_tensor("Wv", [128, NFT * DM], DT_PROJ,
                          kind="ExternalInput")
    Wo_d = nc.dram_tensor("Wo", [128, NFT * DM], DT_ATTN,
                          kind="ExternalInput")
    bias_d = nc.dram_tensor("bias", [DM], F32, kind="ExternalInput")
    mi_d = nc.dram_tensor("maskident", [128, KEYS + 128], DT_ATTN,
                          kind="ExternalInput")
    out_d = nc.dram_tensor("out", [SHARD, DM], F32, kind="ExternalOutput")

    Exp = mybir.ActivationFunctionType.Exp
    Copy = mybir.ActivationFunctionType.Copy
    Mult = mybir.AluOpType.mult

    cp_idx = [0]

    def copy_alt(out, in_):
        # alternate PSUM->SBUF copies between DVE and ACT to balance load
        if cp_idx[0] % 2 == 0:
            nc.vector.tensor_copy(out=out, in_=in_)
        else:
            nc.scalar.activation(out=out, in_=in_, func=Copy)
        cp_idx[0] += 1

    with tile.TileContext(nc) as tc:
        with (
            tc.tile_pool(name="consts", bufs=1) as cpool,
            tc.tile_pool(name="qkv", bufs=1) as qpool,
            tc.tile_pool(name="work", bufs=4) as wpool,
            tc.tile_pool(name="outp", bufs=2) as opool,
            tc.tile_pool(name="pp", bufs=2, space="PSUM") as pp,
            tc.tile_pool(name="ps", bufs=2, space="PSUM") as ps,
            tc.tile_pool(name="pw", bufs=2, space="PSUM") as pw,
        ):
            pa = pw

            # ---- load constants ----
            # split each matrix across two DMAs (two queues) and alternate
            # the issuing engine (only SP and ACT have HWDGE) so descriptor
            # generation (~600ns per dma_start) runs in parallel too
            def load2(dram, cols, dt, tag):
                t = cpool.tile([128, cols], dt, tag=tag)
                h = cols // 2
                nc.sync.dma_start(out=t[:, 0:h], in_=dram[:, 0:h])
                nc.scalar.dma_start(out=t[:, h:cols], in_=dram[:, h:cols])
                return t

            xT_all = load2(xT_d, NFT * HALO, DT_PROJ, "xT")
            Wv_all = load2(Wv_d, NFT * DM, DT_PROJ, "Wv")
            Wq_all = load2(Wq_d, NFT * DM, DT_PROJ, "Wq")
            Wk_all = load2(Wk_d, NFT * DM, DT_PROJ, "Wk")
            Wo_all = load2(Wo_d, NFT * DM, DT_ATTN, "Wo")
            mi_sb = cpool.tile([128, KEYS + 128], DT_ATTN, tag="mi")
            nc.sync.dma_start(out=mi_sb[:], in_=mi_d[:, :])
            bias_sb = cpool.tile([128, DM], F32, tag="bias")
            bias_ap = bias_d[:]
            nc.gpsimd.dma_start(
                out=bias_sb[:],
                in_=bass.AP(tensor=bias_ap.tensor, offset=bias_ap.offset,
                            ap=[[0, 128]] + list(bias_ap.ap)),
            )
            xT_sb = [xT_all[:, kc * HALO:(kc + 1) * HALO]
                     for kc in range(NFT)]
            Wq_sb = [Wq_all[:, kc * DM:(kc + 1) * DM] for kc in range(NFT)]
            Wk_sb = [Wk_all[:, kc * DM:(kc + 1) * DM] for kc in range(NFT)]
            Wv_sb = [Wv_all[:, kc * DM:(kc + 1) * DM] for kc in range(NFT)]
            Wo_sb = [Wo_all[:, kc * DM:(kc + 1) * DM] for kc in range(NFT)]
            mask_sb = mi_sb[:, 0:KEYS]
            ident_sb = mi_sb[:, KEYS:KEYS + 128]

            # ---- v projection (token-major) + shifted copies ----
            v_sb = [None] * 5
            v_dn = [None] * NBLK   # vdn[t][64:128] = v[t][0:64]
            v_64 = [None] * NBLK   # v64[t] = tokens [128t+64, 128t+192)

            def emit_v(tt):
                rows = 128 if tt < 4 else HALO - 512
                psv = pp.tile([128, DM], F32, tag="pp", name="psv")
                for kc in range(NFT):
                    nc.tensor.matmul(
                        psv[:rows, :], xT_sb[kc][:, tt * 128:tt * 128 + rows],
                        Wv_sb[kc][:, :],
                        start=(kc == 0), stop=(kc == NFT - 1))
                vt = qpool.tile([128, DM], DT_ATTN, tag=f"v{tt}",
                                name=f"v{tt}")
                copy_alt(vt[:rows, :], psv[:rows, :])
                v_sb[tt] = vt
                if tt < NBLK:
                    vd = qpool.tile([128, DM], DT_ATTN, tag=f"vdn{tt}",
                                    name=f"vdn{tt}")
                    nc.gpsimd.dma_start(out=vd[64:128, :], in_=vt[0:64, :])
                    v_dn[tt] = vd
                if tt >= 1:
                    v6 = qpool.tile([128, DM], DT_ATTN, tag=f"v64{tt - 1}",
                                    name=f"v64{tt - 1}")
                    nc.gpsimd.dma_start(out=v6[0:64, :],
                                        in_=v_sb[tt - 1][64:128, :])
                    nc.gpsimd.dma_start(out=v6[64:128, :], in_=vt[0:64, :])
                    v_64[tt - 1] = v6

            # ---- qT/kT projection for one head pair (feature tile) ----
            qT_sb = [None] * NFT
            kT_sb = [None] * NFT

            def emit_qk(ft):
                csl = slice(ft * 128, ft * 128 + 128)
                psq = pp.tile([128, SHARD], F32, tag="pp", name="psq")
                for kc in range(NFT):
                    nc.tensor.matmul(
                        psq[:], Wq_sb[kc][:, csl], xT_sb[kc][:, PAD:PAD + SHARD],
                        start=(kc == 0), stop=(kc == NFT - 1))
                qt = qpool.tile([128, SHARD], DT_ATTN, tag=f"qT{ft}",
                                name=f"qT{ft}")
                copy_alt(qt[:], psq[:])
                qT_sb[ft] = qt

                kt = qpool.tile([128, HALO], DT_ATTN, tag=f"kT{ft}",
                                name=f"kT{ft}")
                psk = pp.tile([128, SHARD], F32, tag="pp", name="psk")
                for kc in range(NFT):
                    nc.tensor.matmul(
                        psk[:], Wk_sb[kc][:, csl], xT_sb[kc][:, 0:512],
                        start=(kc == 0), stop=(kc == NFT - 1))
                copy_alt(kt[:, 0:512], psk[:])
                psk2 = pp.tile([128, 64], F32, tag="pp", name="psk2")
                for kc in range(NFT):
                    nc.tensor.matmul(
                        psk2[:], Wk_sb[kc][:, csl], xT_sb[kc][:, 512:HALO],
                        start=(kc == 0), stop=(kc == NFT - 1))
                copy_alt(kt[:, 512:HALO], psk2[:])
                kT_sb[ft] = kt

            attnT_sb = [qpool.tile([128, SHARD], DT_ATTN, tag=f"attnT{i}",
                                   name=f"attnT{i}")
                        for i in range(NFT)]

            # ---- attention stages ----
            # L = [w0 (keys 0:192) | w1 (keys 0:192)] contiguous; transposes:
            #   wT[:,0:128]   = L[:,0:128].T   = h0 keys 0:128
            #   wT[:,128:256] = L[:,128:256].T = h0 keys 128:192 @ p0:64
            #                                  + h1 keys 0:64    @ p64:128
            #   wT[:,256:384] = L[:,256:384].T = h1 keys 64:192
            pendingB = []
            SKEW = 1

            def flushB(n=0):
                while len(pendingB) > n:
                    pendingB.pop(0)()

            def stageA(b, hp):
                qsl = slice(b * 128, b * 128 + 128)
                ksl = slice(b * 128, b * 128 + KEYS)
                denom = wpool.tile([128, 2], F32, tag="denom")
                sc = ps.tile([128, 1024], F32, tag="ps", name="sc")
                for j in range(2):
                    rsl = slice(j * 64, j * 64 + 64)
                    nc.tensor.matmul(
                        sc[:, 512 * j:512 * j + KEYS],
                        qT_sb[hp][rsl, qsl], kT_sb[hp][rsl, ksl],
                        start=True, stop=True)
                e = wpool.tile([128, 2 * KEYS], DT_ATTN, tag="e", name="e")
                sc_view = sc[:].rearrange(
                    "p (h x) -> p h x", h=2)[:, :, 0:KEYS]
                nc.scalar.activation(
                    out=e[:].rearrange("p (h k) -> p h k", h=2),
                    in_=sc_view, func=Exp, scale=0.125)
                ems = []
                for j in range(2):
                    em = wpool.tile([128, KEYS], DT_ATTN, tag="em",
                                    name=f"em{j}")
                    nc.vector.scalar_tensor_tensor(
                        out=em[:], in0=e[:, j * KEYS:(j + 1) * KEYS],
                        scalar=1.0, in1=mask_sb[:],
                        op0=Mult, op1=Mult,
                        accum_out=denom[:, j:j + 1])
                    ems.append(em)
                recip = wpool.tile([128, 2], F32, tag="recip")
                nc.vector.reciprocal(recip[:], denom[:])
                L = wpool.tile([128, 384], DT_ATTN, tag="L", name="L")
                for j in range(2):
                    nc.vector.tensor_scalar_mul(
                        L[:, 192 * j:192 * (j + 1)],
                        ems[j][:], recip[:, j:j + 1])
                return L

            def stageB(b, hp, L):
                qsl = slice(b * 128, b * 128 + 128)
                pwt = pw.tile([128, 384], DT_ATTN, tag="pw", name="pwt")
                nc.tensor.transpose(pwt[:, 0:128], L[:, 0:128], ident_sb[:])
                nc.tensor.transpose(pwt[:, 128:256], L[:, 128:256],
                                    ident_sb[:])
                nc.tensor.transpose(pwt[:, 256:384], L[:, 256:384],
                                    ident_sb[:])
                wT = wpool.tile([128, 384], DT_ATTN, tag="wT", name="wT")
                copy_alt(wT[:], pwt[:])
                h0 = slice(2 * hp * 64, 2 * hp * 64 + 64)
                h1 = slice((2 * hp + 1) * 64, (2 * hp + 1) * 64 + 64)
                pav = pa.tile([128, 128], F32, tag="pw")
                nc.tensor.matmul(pav[0:64, :], v_sb[b][:, h0],
                                 wT[:, 0:128], start=True, stop=False)
                nc.tensor.matmul(pav[0:64, :], v_sb[b + 1][0:64, h0],
                                 wT[0:64, 128:256], start=False, stop=True)
                nc.tensor.matmul(pav[64:128, :], v_dn[b][64:128, h1],
                                 wT[64:128, 128:256], start=True, stop=False)
                nc.tensor.matmul(pav[64:128, :], v_64[b][:, h1],
                                 wT[:, 256:384], start=False, stop=True)
                copy_alt(attnT_sb[hp][:, qsl], pav[:])

            # ---- main schedule ----
            for tt in range(5):
                emit_v(tt)
            emit_qk(0)
            if stage == "proj":
                for ft in range(1, NFT):
                    emit_qk(ft)
                for tt in range(NBLK):
                    osb = opool.tile([128, DM], F32, tag="osb")
                    nc.vector.tensor_copy(osb[:], v_sb[tt][:])
                    nc.sync.dma_start(
                        out=out_d[tt * 128:(tt + 1) * 128, :], in_=osb[:])
            else:
                for hp in range(H // 2):
                    for b in range(NBLK):
                        L = stageA(b, hp)
                        flushB(SKEW - 1)
                        pendingB.append(lambda b=b, hp=hp, L=L:
                                        stageB(b, hp, L))
                        if b == 1 and hp < NFT - 1:
                            emit_qk(hp + 1)
                flushB()
                for b in range(NBLK):
                    qsl = slice(b * 128, b * 128 + 128)
                    po = pp.tile([128, DM], F32, tag="pp")
                    for kc in range(NFT):
                        nc.tensor.matmul(
                            po[:], attnT_sb[kc][:, qsl], Wo_sb[kc][:, :],
                            start=(kc == 0), stop=(kc == NFT - 1))
                    osb = opool.tile([128, DM], F32, tag="osb")
                    nc.vector.tensor_add(osb[:], po[:], bias_sb[:])
                    nc.sync.dma_start(out=out_d[b * 128:(b + 1) * 128, :],
                                      in_=osb[:])

    nc.compile()
    return nc


_CACHE = {}


def _get_program():
    if "nc" not in _CACHE:
        _CACHE["nc"] = _build_program()
    return _CACHE["nc"]


def _make_in_maps(x, W_qkv, W_out, b_out):
    np_proj = _np_dt(DT_PROJ)
    np_attn = _np_dt(DT_ATTN)
    def pack(M, dt):
        # [512, C] -> [128, 4*C] with row-tile kc at cols [kc*C, (kc+1)*C)
        return np.ascontiguousarray(
            np.concatenate([M[128 * kc:128 * (kc + 1)] for kc in range(NFT)],
                           axis=1), dtype=dt)

    Wr = W_qkv.reshape(DM, H, 3, D)
    Wq = pack(Wr[:, :, 0, :].reshape(DM, DM), np_proj)
    Wk = pack(Wr[:, :, 1, :].reshape(DM, DM), np_proj)
    Wv = pack(Wr[:, :, 2, :].reshape(DM, DM), np_proj)
    Wo = pack(W_out, np_attn)
    bias = np.ascontiguousarray(b_out, dtype=np.float32)
    ii = np.arange(128)[:, None]
    kk = np.arange(KEYS)[None, :]
    mask2 = np.where((kk >= ii) & (kk <= ii + WIN - 1), 1.0, 0.0)
    maskident = np.ascontiguousarray(
        np.concatenate([mask2, np.eye(128)], axis=1), dtype=np_attn)

    in_maps = []
    for c in range(NCORES):
        bidx, s0 = c // (NCORES // B), (c % (NCORES // B)) * SHARD
        xh = np.zeros((HALO, DM), np.float32)
        lo, hi = s0 - PAD, s0 + SHARD + PAD
        clo, chi = max(lo, 0), min(hi, S)
        xh[clo - lo:chi - lo] = x[bidx, clo:chi]
        xT = pack(np.ascontiguousarray(xh.T), np_proj)
        in_maps.append({
            "xT": xT, "Wq": Wq, "Wk": Wk, "Wv": Wv, "Wo": Wo,
            "bias": bias, "maskident": maskident,
        })
    return in_maps


def kernel(x, W_qkv, W_out, b_out, _trace=False, _tmpdir=None):
    x = np.asarray(x, dtype=np.float32)
    W_qkv = np.asarray(W_qkv, dtype=np.float32)
    W_out = np.asarray(W_out, dtype=np.float32)
    b_out = np.asarray(b_out, dtype=np.float32)

    nc = _get_program()
    in_maps = _make_in_maps(x, W_qkv, W_out, b_out)
    res = run_bass_kernel_spmd(
        nc, in_maps, list(range(NCORES)), trace=_trace, tmpdir=_tmpdir)
    _CACHE["last_results"] = res
    out = np.concatenate(
        [res.results[c]["out"] for c in range(NCORES)], axis=0)
    return out.reshape(B, S, DM).astype(np.float32)
